# revision 1
# baseline (speedup 1.0000x reference)
"""MQA attention (32 query heads, 1 KV head, ALiBi, causal) on 8 trn2 cores.

Sharding: tensor-parallel over query heads (4 heads/core). Wq rows and Wo
columns are sharded; x, Wk, Wv are replicated. Each core computes a partial
[T, E] output (its 4 heads pushed through its Wo column-shard); the host sums
the 8 partials.

Math per core c (heads g = 4c..4c+3):
  qT_h = (Wq_h * D^-0.5) @ x^T                     [64, T]   (per head)
  kT   = Wk @ x^T                                  [64, T]
  v    = (Wv @ x^T)^T                              [T, 64]
  ST_h[j, i] = kT[:,j].q  +  (-s_h * i)            via augmented row (ones in kT_aug,
                                                    -s_h*i row in qT_aug)
  g = exp(ST + s_h*j)      (per-partition ACT bias; per-column factors cancel
                            in softmax normalization; causal mask via
                            affine_select fill 0 on diagonal blocks)
  OT_aug = [v | 1]^T @ g   -> rows 0:64 = unnormalized head out^T,
                              row 64    = softmax denominator
  headout^T = OT / denom   (partition_broadcast of 1/denom)
  partial = headout^T.T @ WoT_shard                [T, E]

All matmuls run as float32r (TF32-like PE mode, 4x faster than fp32).
Per-column fp32r rounding of the -s_h*i row cancels in normalization.
"""

import numpy as np

import concourse.bacc as bacc
import concourse.bass as bass
import concourse.mybir as mybir
import concourse.tile as tile
from concourse.masks import make_identity
from concourse.bass_utils import run_bass_kernel_spmd

T = 2048          # tokens
E = 2048          # embed dim
H = 32            # query heads
D = 64            # head dim
NCORES = 8
HL = H // NCORES  # 4 heads per core
ES = HL * D       # 256 = per-core E shard
TQ = 512          # moving-operand tile (max fp32 N)
NTQ = T // TQ     # 4
NE = E // 128     # 16 contraction chunks
NT128 = T // 128  # 16

F32 = mybir.dt.float32
F32R = mybir.dt.float32r
EXP = mybir.ActivationFunctionType.Exp

_CACHE = {}


def _build_nc(debug=False):
    nc = bacc.Bacc("TRN2")
    xT = nc.dram_tensor("xT", [E, T], F32R, kind="ExternalInput")
    wqT = nc.dram_tensor("wqT", [E, ES], F32R, kind="ExternalInput")
    wkvT = nc.dram_tensor("wkvT", [E, 2 * D], F32R, kind="ExternalInput")
    woT = nc.dram_tensor("woT", [ES, E], F32R, kind="ExternalInput")
    qrow = nc.dram_tensor("qrow", [HL, T], F32R, kind="ExternalInput")
    ones = nc.dram_tensor("ones", [1, T], F32R, kind="ExternalInput")
    btbl = nc.dram_tensor("btbl", [128, HL * NT128], F32, kind="ExternalInput")
    part = nc.dram_tensor("part", [T, E], F32, kind="ExternalOutput")
    dbg = None
    if debug:
        dbg = {
            "k": nc.dram_tensor("dbg_k", [65, T], F32, kind="ExternalOutput"),
            "v": nc.dram_tensor("dbg_v", [128, NT128, D + 1], F32, kind="ExternalOutput"),
            "otn": nc.dram_tensor("dbg_otn", [2, 128, T], F32, kind="ExternalOutput"),
        }

    from contextlib import ExitStack
    with tile.TileContext(nc) as tc, ExitStack() as ctx:
        _body(nc, tc, ctx, xT, wqT, wkvT, woT, qrow, ones, btbl, part, dbg=dbg)
    nc.finalize()
    return nc


def _body(nc, tc, ctx, xT, wqT, wkvT, woT, qrow, ones, btbl, part, dbg=None):
    const = ctx.enter_context(tc.tile_pool(name="const", bufs=1))
    xtp = ctx.enter_context(tc.tile_pool(name="xt", bufs=20))
    stg = ctx.enter_context(tc.tile_pool(name="stg", bufs=3))
    gp = ctx.enter_context(tc.tile_pool(name="g", bufs=6))
    dnp = ctx.enter_context(tc.tile_pool(name="dn", bufs=2))
    bcp = ctx.enter_context(tc.tile_pool(name="bc", bufs=4))
    osp = ctx.enter_context(tc.tile_pool(name="ostage", bufs=4))

    # ---------- resident constants (weight DMAs emitted inside phase 1) ----
    wq_res = const.tile([128, NE, ES], F32R)
    wkv_res = const.tile([128, NE, 2 * D], F32R)
    wo_res = []
    for p2 in range(2):
        w = const.tile([128, E], F32R, tag=f"wo{p2}")
        wo_res.append(w)
    qTa = []
    for h in range(HL):
        qa = const.tile([65, T], F32R, tag=f"qTa{h}")
        nc.sync.dma_start(out=qa[64:65, :], in_=qrow[h:h + 1, :])
        qTa.append(qa)
    kTa = const.tile([65, T], F32R)
    nc.sync.dma_start(out=kTa[64:65, :], in_=ones[0:1, :])
    v_aug = const.tile([128, NT128, D + 1], F32R)
    ones_col = ones[0:1, 0:NT128]
    ones_bcast = bass.AP(
        tensor=ones_col.tensor, offset=ones_col.offset,
        ap=[[0, 128], [1, NT128]])
    nc.sync.dma_start(out=v_aug[:, :, D:D + 1], in_=ones_bcast)
    btbl_t = const.tile([128, HL * NT128], F32)
    nc.sync.dma_start(out=btbl_t, in_=btbl[:, :])
    ident = const.tile([128, 128], F32)
    make_identity(nc, ident)
    otn = []
    for p2 in range(2):
        o = const.tile([128, T], F32R, tag=f"otn{p2}")
        otn.append(o)

    # ---------- 8 PSUM banks total: pacc 2 + st 2 + ot 2 + out 2 ----------
    pp = ctx.enter_context(tc.tile_pool(name="pacc", bufs=2, space="PSUM"))
    stp = ctx.enter_context(tc.tile_pool(name="st_ps", bufs=3, space="PSUM"))
    otp = ctx.enter_context(tc.tile_pool(name="ot_ps", bufs=2, space="PSUM"))
    oup = ctx.enter_context(tc.tile_pool(name="out_ps", bufs=1, space="PSUM"))

    for q in range(NTQ):
        _projections_q(nc, q, pp, stp, xtp, stg, xT, wqT, wkvT, woT,
                       wq_res, wkv_res, wo_res, qTa, kTa, v_aug, ident)
        for h in range(HL):
            _attention_hq(nc, h, q, stp, otp, gp, dnp, bcp, stg,
                          qTa, kTa, v_aug, btbl_t, otn)
        for t in range(4 * q, 4 * q + 4):
            _outproj_t(nc, t, oup, stp, osp, otn, wo_res, part)
    if dbg is not None:
        nc.sync.dma_start(out=dbg["k"][:, :], in_=kTa[:, :].bitcast(F32))
        nc.sync.dma_start(out=dbg["v"][:, :, :], in_=v_aug[:, :, :].bitcast(F32))
        for p2 in range(2):
            nc.sync.dma_start(out=dbg["otn"][p2], in_=otn[p2][:, :].bitcast(F32))


def _projections_q(nc, q, pp, stp, xtp, stg, xT, wqT, wkvT, woT,
                   wq_res, wkv_res, wo_res, qTa, kTa, v_aug, ident):
    cs, ce = q * TQ, (q + 1) * TQ
    xts = []
    for e in range(NE):
        if q == 0:
            nc.sync.dma_start(out=wq_res[:, e, :],
                              in_=wqT[e * 128:(e + 1) * 128, :])
            nc.sync.dma_start(out=wkv_res[:, e, :],
                              in_=wkvT[e * 128:(e + 1) * 128, :])
        xt = xtp.tile([128, TQ], F32R, tag="xt")
        eng = nc.gpsimd if q == 0 else nc.sync
        eng.dma_start(out=xt, in_=xT[e * 128:(e + 1) * 128, cs:ce])
        xts.append(xt)
    # group 0: heads 0/1
    acc = pp.tile([128, TQ], F32, tag="acc")
    for e in range(NE):
        nc.tensor.matmul(acc, wq_res[:, e, 0:128], xts[e],
                         start=(e == 0), stop=(e == NE - 1))
    nc.vector.tensor_copy(out=qTa[0][0:64, cs:ce], in_=acc[0:64, :])
    st0 = stg.tile([128, TQ], F32R, tag="stq")
    nc.vector.tensor_copy(out=st0[64:128, :], in_=acc[64:128, :])
    nc.sync.dma_start(out=qTa[1][0:64, cs:ce], in_=st0[64:128, :])
    # group 1: heads 2/3
    acc = pp.tile([128, TQ], F32, tag="acc")
    for e in range(NE):
        nc.tensor.matmul(acc, wq_res[:, e, 128:256], xts[e],
                         start=(e == 0), stop=(e == NE - 1))
    nc.vector.tensor_copy(out=qTa[2][0:64, cs:ce], in_=acc[0:64, :])
    st1 = stg.tile([128, TQ], F32R, tag="stq")
    nc.vector.tensor_copy(out=st1[64:128, :], in_=acc[64:128, :])
    nc.sync.dma_start(out=qTa[3][0:64, cs:ce], in_=st1[64:128, :])
    # group 2: k (rows 0:64) and v (rows 64:128)
    acc = pp.tile([128, TQ], F32, tag="acc")
    for e in range(NE):
        nc.tensor.matmul(acc, wkv_res[:, e, :], xts[e],
                         start=(e == 0), stop=(e == NE - 1))
    nc.vector.tensor_copy(out=kTa[0:64, cs:ce], in_=acc[0:64, :])
    stv = stg.tile([128, TQ], F32R, tag="stv")
    nc.vector.tensor_copy(out=stv[64:128, :], in_=acc[64:128, :])
    # v transpose via PE: 4x [64, 128] -> [128, 64]
    for mm in range(4):
        tr = stp.tile([128, TQ], F32, tag="st")
        nc.tensor.transpose(
            tr[:, 0:D],
            stv[64:128, mm * 128:(mm + 1) * 128].bitcast(F32),
            ident[64:128, 64:128])
        nc.vector.tensor_copy(out=v_aug[:, q * 4 + mm, 0:D], in_=tr[:, 0:D])
    if q == 0:
        for p2 in range(2):
            nc.sync.dma_start(out=wo_res[p2],
                              in_=woT[p2 * 128:(p2 + 1) * 128, :])

def _attention_hq(nc, h, q, stp, otp, gp, dnp, bcp, stg,
                  qTa, kTa, v_aug, btbl_t, otn):
    cs = q * TQ
    ntk = 4 * q + 4
    ot = otp.tile([65, TQ], F32, tag="ot")
    for tk in range(ntk):
        m = tk - 4 * q
        # pad narrow diag tiles to >=256 cols (fp32r 1 cyc/row zone)
        lo = min(max(0, m * 128), TQ - 256)
        st = stp.tile([128, TQ], F32, tag="st")
        nc.tensor.matmul(
            st[:, lo:TQ],
            kTa[:, tk * 128:(tk + 1) * 128],
            qTa[h][:, cs + lo:cs + TQ],
            start=True, stop=True,
        )
        g = gp.tile([128, TQ], F32R, tag="g")
        nc.scalar.activation(
            out=g[:, lo:TQ], in_=st[:, lo:TQ], func=EXP,
            bias=btbl_t[:, h * NT128 + tk:h * NT128 + tk + 1],
            scale=1.0,
        )
        if m >= 0:
            mw = m * 128 + 128 - lo
            nc.gpsimd.affine_select(
                out=g[:, lo:lo + mw], in_=g[:, lo:lo + mw],
                compare_op=mybir.AluOpType.is_ge,
                fill=0.0, base=lo - m * 128,
                pattern=[[1, mw]], channel_multiplier=-1,
            )
        nc.tensor.matmul(
            ot[:, lo:TQ], v_aug[:, tk, :], g[:, lo:TQ],
            start=(tk == 0), stop=(tk == ntk - 1),
        )
    # normalize: rows 0:64 / row 64
    dn = dnp.tile([65, TQ], F32, tag="dn")
    nc.vector.tensor_copy(out=dn[64:65, :], in_=ot[64:65, :])
    dn0 = dnp.tile([1, TQ], F32, tag="dn0")
    nc.sync.dma_start(out=dn0[0:1, :], in_=dn[64:65, :])
    rc = dnp.tile([1, TQ], F32, tag="rc")
    nc.vector.reciprocal(out=rc[0:1, :], in_=dn0[0:1, :])
    bc = bcp.tile([64, TQ], F32)
    nc.gpsimd.partition_broadcast(bc, rc[0:1, :])
    pair, half = h // 2, h % 2
    if half == 0:
        nc.vector.tensor_mul(
            out=otn[pair][0:64, cs:cs + TQ], in0=ot[0:64, :], in1=bc)
    else:
        so = stg.tile([128, TQ], F32R, tag="sot")
        nc.vector.tensor_mul(out=so[0:64, :], in0=ot[0:64, :], in1=bc)
        nc.sync.dma_start(
            out=otn[pair][64:128, cs:cs + TQ], in_=so[0:64, :])

def _outproj_t(nc, t, oup, stp, osp, otn, wo_res, part):
    for o in range(NTQ):
        if t >= 12 and o % 2 == 1:
            po = stp.tile([128, TQ], F32, tag="st")
        else:
            po = oup.tile([128, TQ], F32, tag="po")
        nc.tensor.matmul(
            po, otn[0][:, t * 128:(t + 1) * 128],
            wo_res[0][:, o * TQ:(o + 1) * TQ], start=True, stop=False)
        nc.tensor.matmul(
            po, otn[1][:, t * 128:(t + 1) * 128],
            wo_res[1][:, o * TQ:(o + 1) * TQ], start=False, stop=True)
        ob = osp.tile([128, TQ], F32)
        if t >= 12:
            nc.vector.tensor_copy(out=ob[:, 0:256], in_=po[:, 0:256])
            nc.scalar.copy(out=ob[:, 256:TQ], in_=po[:, 256:TQ])
        else:
            nc.vector.tensor_copy(out=ob, in_=po)
        nc.sync.dma_start(
            out=part[t * 128:(t + 1) * 128, o * TQ:(o + 1) * TQ], in_=ob)


def _prepare_in_maps(x, Wq, Wk, Wv, Wo):
    xTn = np.ascontiguousarray(x[0].T).astype(np.float32)
    wkvTn = np.ascontiguousarray(np.concatenate([Wk, Wv], axis=0).T).astype(np.float32)
    scale = np.float32(D ** -0.5)
    in_maps = []
    for c in range(NCORES):
        sl = slice(c * ES, (c + 1) * ES)
        wqTn = np.ascontiguousarray((Wq[sl, :] * scale).T).astype(np.float32)
        woTn = np.ascontiguousarray(Wo[:, sl].T).astype(np.float32)
        g = np.arange(c * HL, (c + 1) * HL, dtype=np.float64)
        slopes = np.power(2.0, -8.0 * (g + 1.0) / H)  # [HL]
        i = np.arange(T, dtype=np.float64)
        qrow_n = (-slopes[:, None] * i[None, :]).astype(np.float32)  # [HL, T]
        p = np.arange(128, dtype=np.float64)
        k = np.arange(NT128, dtype=np.float64)
        # btbl[p, h*16+k] = s_h * (k*128 + p)
        btbl_n = (slopes[:, None, None] * (k[None, :, None] * 128 + p[None, None, :]))
        btbl_n = np.ascontiguousarray(
            btbl_n.transpose(2, 0, 1).reshape(128, HL * NT128)).astype(np.float32)
        in_maps.append({
            "xT": xTn, "wqT": wqTn, "wkvT": wkvTn, "woT": woTn,
            "qrow": qrow_n, "ones": np.ones((1, T), dtype=np.float32),
            "btbl": btbl_n,
        })
    return in_maps


def kernel(x, Wq, Wk, Wv, Wo, attention_mask, _trace=False, _trace_cores=None):
    x = np.asarray(x, dtype=np.float32)
    Wq = np.asarray(Wq, dtype=np.float32)
    Wk = np.asarray(Wk, dtype=np.float32)
    Wv = np.asarray(Wv, dtype=np.float32)
    Wo = np.asarray(Wo, dtype=np.float32)

    if "nc" not in _CACHE:
        _CACHE["nc"] = _build_nc()
    nc = _CACHE["nc"]

    in_maps = _prepare_in_maps(x, Wq, Wk, Wv, Wo)
    kwargs = {}
    if _trace:
        kwargs = {"trace": True, "trace_cores": _trace_cores or [0]}
    res = run_bass_kernel_spmd(nc, in_maps, core_ids=list(range(NCORES)), **kwargs)
    acc = np.zeros((T, E), dtype=np.float64)
    for r in res.results:
        acc += r["part"].astype(np.float64)
    out = acc.astype(np.float32)[None, :, :]
    if _trace:
        _CACHE["last_result"] = res
    return out



# revision 3
# speedup vs baseline: 1.0101x; 1.0101x over previous
"""MQA attention (32 query heads, 1 KV head, ALiBi, causal) on 8 trn2 cores.

Sharding: tensor-parallel over query heads (4 heads/core). Wq rows and Wo
columns are sharded; x, Wk, Wv are replicated. Each core computes a partial
[T, E] output (its 4 heads through its Wo column-shard) in bf16; the host
sums the 8 partials in fp32.

All matmuls run in bf16 (1 cycle/row on PE regardless of width). Per core:

  proj (PE):   qT_h = (Wq_h D^-.5) @ x^T [64,T]/head, kvT = Wkv @ x^T [128,T]
  scores (PE): ST[k,q] = kTa[:,kblk]^T qTa[:,qcols]  (aug row: ones x -s_h i)
  exp (ACT):   g = exp(ST + s_h(128 tk + p))  bias per (head, k-tile)
  mask (Pool): affine_select zeroes j>i on diagonal 128-blocks
  AV (PE):     av[128q, 65] += g_blk[128k,128q]^T @ v_aug[128k,65]
               (col 64 of v_aug is ones -> softmax denominator)
  norm (DVE):  hsb[p, m, d] = av[p,m,d] * recip(av[p,m,64])
  transpose:   DMA-xbar hsb -> otn[d, qcols] (headout^T, per (h,q))
  Wo (PE):     part[t,:] = otn0[:,tblk]^T wo0 + otn1[:,tblk]^T wo1

The -s_h*i aug row cancels exactly in softmax normalization (per-query
constant), so bf16 rounding of it is harmless. Emission interleaves three
streams (proj / scores+AV / Wo) with a proportional merge so the PE stays
fed while ACT drains the exps.
"""

import numpy as np
import ml_dtypes

import concourse.bacc as bacc
import concourse.bass as bass
import concourse.mybir as mybir
import concourse.tile as tile
from concourse.bass_utils import run_bass_kernel_spmd

T = 2048
E = 2048
H = 32
D = 64
NCORES = 8
HL = H // NCORES   # 4 heads per core
ES = HL * D        # 256 per-core E shard
TQ = 512           # query tile
NTQ = T // TQ      # 4
NE = E // 128      # 16 contraction chunks
NT128 = T // 128   # 16

F32 = mybir.dt.float32
BF16 = mybir.dt.bfloat16
EXP = mybir.ActivationFunctionType.Exp

_CACHE = {}

# merge weights: higher -> stream finishes earlier in the emission
W_PROJ = 2.4
W_SA = 1.0
W_WO = 0.66


def _merge(streams):
    """streams: list of (steps, weight); steps = list of (cost, fn).
    Emits every fn once, proportionally by weighted cumulative cost."""
    totals = [max(sum(c for c, _ in s), 1e-9) * w for s, w in streams]
    done = [0.0] * len(streams)
    idx = [0] * len(streams)
    while True:
        best, bestv = -1, None
        for i, (s, _w) in enumerate(streams):
            if idx[i] >= len(s):
                continue
            v = done[i] / totals[i]
            if best < 0 or v < bestv:
                best, bestv = i, v
        if best < 0:
            return
        c, fn = streams[best][0][idx[best]]
        idx[best] += 1
        done[best] += c
        fn()


def _build_nc():
    nc = bacc.Bacc("TRN2")
    xT = nc.dram_tensor("xT", [E, T], BF16, kind="ExternalInput")
    wqT = nc.dram_tensor("wqT", [E, ES], BF16, kind="ExternalInput")
    wkvT = nc.dram_tensor("wkvT", [E, 2 * D], BF16, kind="ExternalInput")
    woT = nc.dram_tensor("woT", [ES, E], BF16, kind="ExternalInput")
    qaug = nc.dram_tensor("qaug", [HL, T], BF16, kind="ExternalInput")
    ones = nc.dram_tensor("ones", [1, T], BF16, kind="ExternalInput")
    btbl = nc.dram_tensor("btbl", [128, HL * NT128], F32, kind="ExternalInput")
    part = nc.dram_tensor("part", [T, E], BF16, kind="ExternalOutput")

    from contextlib import ExitStack
    with tile.TileContext(nc) as tc, ExitStack() as ctx:
        _body(nc, tc, ctx, xT, wqT, wkvT, woT, qaug, ones, btbl, part)
    nc.finalize()
    return nc


def _body(nc, tc, ctx, xT, wqT, wkvT, woT, qaug, ones, btbl, part):
    const = ctx.enter_context(tc.tile_pool(name="const", bufs=1))
    xtp = ctx.enter_context(tc.tile_pool(name="xt", bufs=1))
    gp = ctx.enter_context(tc.tile_pool(name="g", bufs=1))
    sp = ctx.enter_context(tc.tile_pool(name="stage", bufs=2))
    obp = ctx.enter_context(tc.tile_pool(name="ob", bufs=2))
    ps = ctx.enter_context(tc.tile_pool(name="ps", bufs=1, space="PSUM"))

    # ---------------- resident constants ----------------
    wq_res = const.tile([128, NE, ES], BF16)
    nc.sync.dma_start(out=wq_res,
                      in_=wqT[:, :].rearrange("(e p) m -> p e m", p=128))
    wkv_res = const.tile([128, NE, 2 * D], BF16)
    nc.sync.dma_start(out=wkv_res,
                      in_=wkvT[:, :].rearrange("(e p) m -> p e m", p=128))
    wo_res = []
    for p2 in range(2):
        w = const.tile([128, E], BF16, tag=f"wo{p2}")
        nc.sync.dma_start(out=w, in_=woT[p2 * 128:(p2 + 1) * 128, :])
        wo_res.append(w)
    qTa = []
    for h in range(HL):
        qa = const.tile([65, T], BF16, tag=f"qTa{h}")
        nc.sync.dma_start(out=qa[64:65, :], in_=qaug[h:h + 1, :])
        qTa.append(qa)
    kTa = const.tile([65, T], BF16)
    nc.sync.dma_start(out=kTa[64:65, :], in_=ones[0:1, :])
    # v_aug: [128 keys, k-tile, 128 slot]; col 64 = ones (denominator row)
    v_aug = const.tile([128, NT128, 128], BF16)
    ones_col = ones[0:1, 0:NT128]
    ones_bcast = bass.AP(tensor=ones_col.tensor, offset=ones_col.offset,
                         ap=[[0, 128], [1, NT128]])
    nc.sync.dma_start(out=v_aug[:, :, 64:65], in_=ones_bcast)
    btbl_t = const.tile([128, HL * NT128], F32)
    nc.sync.dma_start(out=btbl_t, in_=btbl[:, :])
    otn = []
    for p2 in range(2):
        o = const.tile([128, T], BF16, tag=f"otn{p2}")
        otn.append(o)

    # live tiles, stashed at emission time by the creating step
    live = {}

    # ---------------- proj stream ----------------
    def proj_steps():
        steps = []
        for q in range(NTQ):
            cs, ce = q * TQ, (q + 1) * TQ

            def load_x(q=q, cs=cs, ce=ce):
                xt = xtp.tile([128, NE, TQ], BF16, tag="xt", name=f"xt{q}")
                live[("xt", q)] = xt
                nc.sync.dma_start(
                    out=xt,
                    in_=xT[:, cs:ce].rearrange("(e p) t -> p e t", p=128))
            steps.append((0.0, load_x))

            for grp in range(3):
                for e in range(NE):
                    def mm(q=q, grp=grp, e=e):
                        if e == 0:
                            live[("acc", q, grp)] = ps.tile(
                                [128, TQ], F32, tag="acc", bufs=2,
                                name=f"acc{q}_{grp}")
                        acc = live[("acc", q, grp)]
                        if grp < 2:
                            lhs = wq_res[:, e, grp * 128:(grp + 1) * 128]
                        else:
                            lhs = wkv_res[:, e, :]
                        nc.tensor.matmul(acc, lhs, live[("xt", q)][:, e, :],
                                         start=(e == 0), stop=(e == NE - 1))
                    steps.append((213.0, mm))

                def stage(q=q, grp=grp, cs=cs, ce=ce):
                    acc = live.pop(("acc", q, grp))
                    if grp < 2:
                        nc.vector.tensor_copy(
                            out=qTa[2 * grp][0:64, cs:ce], in_=acc[0:64, :])
                        nc.vector.tensor_copy(
                            out=qTa[2 * grp + 1][0:64, cs:ce],
                            in_=acc[64:128, :])
                    else:
                        nc.vector.tensor_copy(
                            out=kTa[0:64, cs:ce], in_=acc[0:64, :])
                        stv = sp.tile([64, TQ], BF16, tag="stv",
                                      name=f"stv{q}")
                        nc.vector.tensor_copy(out=stv, in_=acc[64:128, :])
                        for m in range(4):
                            nc.sync.dma_start_transpose(
                                out=v_aug[:, 4 * q + m, 0:64],
                                in_=stv[:, m * 128:(m + 1) * 128])
                steps.append((0.0, stage))
        return steps

    # ---------------- scores + AV stream (interleaved per q) -----------
    def score_steps_hq(q, h):
        steps = []
        for tk in range(4 * q + 4):
            qs = max(q * TQ, tk * 128)
            n = (q + 1) * TQ - qs

            def step(h=h, q=q, tk=tk, qs=qs, n=n):
                st = ps.tile([128, TQ], F32, tag="st", bufs=4,
                             name=f"st{q}_{h}_{tk}")
                g = gp.tile([128, TQ], BF16, tag=f"g{h}_{tk}",
                            name=f"g{q}_{h}_{tk}")
                live[("g", h, tk)] = (q, g)
                nc.tensor.matmul(
                    st[:, 0:n],
                    kTa[:, tk * 128:(tk + 1) * 128],
                    qTa[h][:, qs:qs + n],
                    start=True, stop=True)
                nc.scalar.activation(
                    out=g[:, 0:n], in_=st[:, 0:n], func=EXP,
                    bias=btbl_t[:, h * NT128 + tk:h * NT128 + tk + 1],
                    scale=1.0)
                if tk >= 4 * q:
                    nc.gpsimd.affine_select(
                        out=g[:, 0:128], in_=g[:, 0:128],
                        compare_op=mybir.AluOpType.is_ge,
                        fill=0.0, base=0,
                        pattern=[[1, 128]], channel_multiplier=-1)
            steps.append((n * 0.4167, step))
        return steps

    def av_steps_hq(q, h):
        steps = []
        for j in range(4):
            ntk = 4 * q + j + 1

            def mmj(h=h, q=q, j=j, ntk=ntk):
                if j == 0:
                    live[("av", h)] = ps.tile(
                        [128, 4, 65], F32, tag="av", bufs=2,
                        name=f"av{q}_{h}")
                av = live[("av", h)]
                for tk in range(ntk):
                    qs = max(q * TQ, tk * 128)
                    off = q * TQ + j * 128 - qs
                    gq, g = live[("g", h, tk)]
                    assert gq == q, f"stale g tile {gq} != {q}"
                    nc.tensor.matmul(
                        av[:, j, :],
                        g[:, off:off + 128],
                        v_aug[:, tk, 0:65],
                        start=(tk == 0), stop=(tk == ntk - 1))
            steps.append((ntk * 65 * 0.4167, mmj))

        def norm(h=h, q=q):
            av = live.pop(("av", h))
            rc = sp.tile([128, 4], F32, tag=f"rc{h}", name=f"rc{q}{h}")
            nc.vector.reciprocal(out=rc, in_=av[:, :, 64])
            hsb = sp.tile([128, 4, 128], BF16, tag=f"hsb{h}",
                          name=f"hsb{q}{h}")
            rc_b = bass.AP(tensor=rc.tensor, offset=rc.offset,
                           ap=[rc.ap[0], [1, 4], [0, 64]])
            nc.vector.tensor_mul(
                out=hsb[:, :, 0:64], in0=av[:, :, 0:64], in1=rc_b)
            # xbar: otn[pair][64*(h%2)+d, q*TQ + m*128 + p] = hsb[p, m, d]
            osl = otn[h // 2][(h % 2) * 64:(h % 2) * 64 + 64,
                              q * TQ:(q + 1) * TQ]
            oap = bass.AP(tensor=osl.tensor, offset=osl.offset,
                          ap=[osl.ap[0], [128, 4], [1, 128]])
            nc.sync.dma_start_transpose(out=oap, in_=hsb[:, :, :])
        steps.append((0.0, norm))
        return steps

    def sa_steps():
        steps = []
        for q in range(NTQ):
            s = [score_steps_hq(q, h) for h in range(HL)]
            a = [av_steps_hq(q, h) for h in range(HL)]
            # lag AV by ~2.5 heads behind scores to let ACT drain
            steps += s[0] + s[1] + s[2] + a[0] + s[3] + a[1] + a[2] + a[3]
        return steps

    # ---------------- Wo stream ----------------
    def wo_steps():
        steps = []
        for t in range(NT128):
            for o in range(4):
                def mmo(t=t, o=o):
                    if o == 0:
                        live[("ob", t)] = obp.tile(
                            [128, E], BF16, tag="ob", name=f"ob{t}")
                    ob = live[("ob", t)]
                    po = ps.tile([128, TQ], F32, tag="acc", bufs=2,
                                 name=f"po{t}_{o}")
                    nc.tensor.matmul(
                        po, otn[0][:, t * 128:(t + 1) * 128],
                        wo_res[0][:, o * TQ:(o + 1) * TQ],
                        start=True, stop=False)
                    nc.tensor.matmul(
                        po, otn[1][:, t * 128:(t + 1) * 128],
                        wo_res[1][:, o * TQ:(o + 1) * TQ],
                        start=False, stop=True)
                    eng = (nc.vector, nc.gpsimd, nc.vector, nc.gpsimd)[o]
                    eng.tensor_copy(out=ob[:, o * TQ:(o + 1) * TQ], in_=po)
                    if o == 3:
                        live.pop(("ob", t))
                        nc.sync.dma_start(
                            out=part[t * 128:(t + 1) * 128, :], in_=ob)
                steps.append((427.0, mmo))
        return steps

    sP = proj_steps()
    sSA = sa_steps()
    sW = wo_steps()

    # bootstrap: first proj q-tile fully, then merge the rest
    nboot = 1 + 3 * (NE + 1)
    for _c, fn in sP[:nboot]:
        fn()
    _merge([(sP[nboot:], W_PROJ), (sSA, W_SA), (sW, W_WO)])


def _prepare_in_maps(x, Wq, Wk, Wv, Wo):
    bf = ml_dtypes.bfloat16
    xTn = np.ascontiguousarray(x[0].T).astype(bf)
    wkvTn = np.ascontiguousarray(
        np.concatenate([Wk, Wv], axis=0).T).astype(bf)
    scale = np.float64(D) ** -0.5
    in_maps = []
    for c in range(NCORES):
        sl = slice(c * ES, (c + 1) * ES)
        wqTn = np.ascontiguousarray(
            (Wq[sl, :].astype(np.float64) * scale).T).astype(bf)
        woTn = np.ascontiguousarray(Wo[:, sl].T).astype(bf)
        g = np.arange(c * HL, (c + 1) * HL, dtype=np.float64)
        slopes = np.power(2.0, -8.0 * (g + 1.0) / H)          # [HL]
        i = np.arange(T, dtype=np.float64)
        qaug_n = (-slopes[:, None] * i[None, :]).astype(bf)   # [HL, T]
        p = np.arange(128, dtype=np.float64)
        k = np.arange(NT128, dtype=np.float64)
        btbl_n = (slopes[:, None, None]
                  * (k[None, :, None] * 128 + p[None, None, :]))
        btbl_n = np.ascontiguousarray(
            btbl_n.transpose(2, 0, 1).reshape(128, HL * NT128)
        ).astype(np.float32)
        in_maps.append({
            "xT": xTn, "wqT": wqTn, "wkvT": wkvTn, "woT": woTn,
            "qaug": qaug_n, "ones": np.ones((1, T), dtype=bf),
            "btbl": btbl_n,
        })
    return in_maps


def kernel(x, Wq, Wk, Wv, Wo, attention_mask, _trace=False, _trace_cores=None):
    x = np.asarray(x, dtype=np.float32)
    Wq = np.asarray(Wq, dtype=np.float32)
    Wk = np.asarray(Wk, dtype=np.float32)
    Wv = np.asarray(Wv, dtype=np.float32)
    Wo = np.asarray(Wo, dtype=np.float32)

    if "nc" not in _CACHE:
        _CACHE["nc"] = _build_nc()
    nc = _CACHE["nc"]

    in_maps = _prepare_in_maps(x, Wq, Wk, Wv, Wo)
    kwargs = {}
    if _trace:
        kwargs = {"trace": True, "trace_cores": _trace_cores or [0]}
    res = run_bass_kernel_spmd(nc, in_maps, core_ids=list(range(NCORES)),
                               **kwargs)
    acc = np.zeros((T, E), dtype=np.float32)
    for r in res.results:
        acc += np.asarray(r["part"]).astype(np.float32)
    out = acc[None, :, :]
    if _trace:
        _CACHE["last_result"] = res
    return out


# revision 16
# speedup vs baseline: 1.1210x; 1.1098x over previous
"""MQA attention (32 query heads, 1 KV head, ALiBi, causal) on 8 trn2 cores.

Sharding: tensor-parallel over query heads (4 heads/core). Wq rows and Wo
columns are sharded; x, Wk, Wv are replicated. Each core computes a partial
[T, E] output (its 4 heads through its Wo column-shard) in bf16; the host
sums the 8 partials in fp32.

All matmuls run in bf16 (1 cycle/row on PE regardless of width). Per core:

  proj (PE):   qT_h = (Wq_h D^-.5) @ x^T [64,T]/head, kvT = Wkv @ x^T [128,T]
  scores (PE): ST[k,q] = kTa[:,kblk]^T qTa[:,qcols]  (aug row: ones x -s_h i)
  exp (ACT):   g = exp(ST + s_h(128 tk + p))  bias per (head, k-tile)
  mask (Pool): affine_select zeroes j>i on diagonal 128-blocks
  AV (PE):     av[128q, 65] += g_blk[128k,128q]^T @ v_aug[128k,65]
               (col 64 of v_aug is ones -> softmax denominator)
  norm (DVE):  hsb[p, m, d] = av[p,m,d] * recip(av[p,m,64])
  transpose:   DMA-xbar hsb -> otn[d, qcols] (headout^T, per (h,q))
  Wo (PE):     part[t,:] = otn0[:,tblk]^T wo0 + otn1[:,tblk]^T wo1

The -s_h*i aug row cancels exactly in softmax normalization (per-query
constant), so bf16 rounding of it is harmless. Emission interleaves three
streams (proj / scores+AV / Wo) with a proportional merge so the PE stays
fed while ACT drains the exps.
"""

import numpy as np
import ml_dtypes

import concourse.bacc as bacc
import concourse.bass as bass
import concourse.mybir as mybir
import concourse.tile as tile
from concourse.bass_utils import run_bass_kernel_spmd

T = 2048
E = 2048
H = 32
D = 64
NCORES = 8
HL = H // NCORES   # 4 heads per core
ES = HL * D        # 256 per-core E shard
TQ = 512           # query tile
NTQ = T // TQ      # 4
NE = E // 128      # 16 contraction chunks
NT128 = T // 128   # 16

F32 = mybir.dt.float32
BF16 = mybir.dt.bfloat16
EXP = mybir.ActivationFunctionType.Exp

_CACHE = {}

# merge weights: higher -> stream finishes earlier in the emission
W_PROJ = 2.6
W_SA = 1.0


def _merge(streams, collect=None):
    """streams: list of (steps, weight); steps = list of (cost, fn).
    Emits every fn once (or appends to `collect`), proportionally by
    weighted cumulative cost."""
    totals = [max(sum(c for c, _ in s), 1e-9) * w for s, w in streams]
    done = [0.0] * len(streams)
    idx = [0] * len(streams)
    while True:
        best, bestv = -1, None
        for i, (s, _w) in enumerate(streams):
            if idx[i] >= len(s):
                continue
            v = done[i] / totals[i]
            if best < 0 or v < bestv:
                best, bestv = i, v
        if best < 0:
            return
        step = streams[best][0][idx[best]]
        idx[best] += 1
        done[best] += step[0]
        if collect is not None:
            collect.append(step)
        else:
            step[1]()


def _build_nc():
    nc = bacc.Bacc("TRN2")
    xT = nc.dram_tensor("xT", [E, T], BF16, kind="ExternalInput")
    wqT = nc.dram_tensor("wqT", [E, ES], BF16, kind="ExternalInput")
    wkvT = nc.dram_tensor("wkvT", [E, 2 * D], BF16, kind="ExternalInput")
    woT = nc.dram_tensor("woT", [ES, E], BF16, kind="ExternalInput")
    qaug = nc.dram_tensor("qaug", [HL, T], BF16, kind="ExternalInput")
    ones = nc.dram_tensor("ones", [1, T], BF16, kind="ExternalInput")
    btbl = nc.dram_tensor("btbl", [128, HL * NT128], F32, kind="ExternalInput")
    part = nc.dram_tensor("part", [T, E], BF16, kind="ExternalOutput")

    from contextlib import ExitStack
    with tile.TileContext(nc) as tc, ExitStack() as ctx:
        _body(nc, tc, ctx, xT, wqT, wkvT, woT, qaug, ones, btbl, part)
    nc.finalize()
    return nc


def _body(nc, tc, ctx, xT, wqT, wkvT, woT, qaug, ones, btbl, part):
    const = ctx.enter_context(tc.tile_pool(name="const", bufs=1))
    xtp = ctx.enter_context(tc.tile_pool(name="xt", bufs=2))
    gp = ctx.enter_context(tc.tile_pool(name="g", bufs=1))
    sp = ctx.enter_context(tc.tile_pool(name="stage", bufs=2))
    obp = ctx.enter_context(tc.tile_pool(name="ob", bufs=2))
    ps = ctx.enter_context(tc.tile_pool(name="ps", bufs=1, space="PSUM"))

    # ---------------- resident constants (DMAs emitted in bootstrap) ----
    wq_res = const.tile([128, NE, ES], BF16)
    wkv_res = const.tile([128, NE, 2 * D], BF16)
    wo_res = [const.tile([128, E], BF16, tag=f"wo{p2}", name=f"wo{p2}")
              for p2 in range(2)]
    qTa = [const.tile([65, T], BF16, tag=f"qTa{h}", name=f"qTa{h}")
           for h in range(HL)]
    kTa = const.tile([65, T], BF16)
    # v_aug: [128 keys, k-tile, 128 slot]; col 64 = ones (denominator row)
    v_aug = const.tile([128, NT128, 128], BF16)
    btbl_t = const.tile([128, HL * NT128], F32)
    otn = [const.tile([128, T], BF16, tag=f"otn{p2}", name=f"otn{p2}")
           for p2 in range(2)]

    # live tiles, stashed at emission time by the creating step
    live = {}

    def load_consts_early():
        # wq + xt(0) interleaved in 2-chunk pieces so P(0) starts ~2us in
        xt0 = xtp.tile([128, NE, TQ], BF16, tag="xt", name="xt0")
        live[("xt", 0)] = xt0
        for e2 in range(NE // 2):
            sl = slice(e2 * 256, (e2 + 1) * 256)
            nc.sync.dma_start(
                out=wq_res[:, 2 * e2:2 * e2 + 2, :],
                in_=wqT[sl, :].rearrange("(e p) m -> p e m", p=128))
            nc.sync.dma_start(
                out=xt0[:, 2 * e2:2 * e2 + 2, :],
                in_=xT[sl, 0:TQ].rearrange("(e p) t -> p e t", p=128))
        nc.sync.dma_start(out=wkv_res,
                          in_=wkvT[:, :].rearrange("(e p) m -> p e m", p=128))
        for h in range(HL):
            nc.sync.dma_start(out=qTa[h][64:65, :], in_=qaug[h:h + 1, :])
        nc.sync.dma_start(out=kTa[64:65, :], in_=ones[0:1, :])
        ones_col = ones[0:1, 0:NT128]
        ones_bcast = bass.AP(tensor=ones_col.tensor, offset=ones_col.offset,
                             ap=[[0, 128], [1, NT128]])
        nc.sync.dma_start(out=v_aug[:, :, 64:65], in_=ones_bcast)
        nc.sync.dma_start(out=btbl_t, in_=btbl[:, :])

    def load_consts_late():
        for p2 in range(2):
            nc.sync.dma_start(out=wo_res[p2],
                              in_=woT[p2 * 128:(p2 + 1) * 128, :])

    # ---------------- proj stream ----------------
    def proj_steps():
        steps = []
        for q in range(NTQ):
            cs, ce = q * TQ, (q + 1) * TQ

            if q > 0:
                def load_x(q=q, cs=cs, ce=ce):
                    xt = xtp.tile([128, NE, TQ], BF16, tag="xt",
                                  name=f"xt{q}")
                    live[("xt", q)] = xt
                    for e4 in range(4):
                        sl = slice(e4 * 512, (e4 + 1) * 512)
                        nc.sync.dma_start(
                            out=xt[:, 4 * e4:4 * e4 + 4, :],
                            in_=xT[sl, cs:ce].rearrange(
                                "(e p) t -> p e t", p=128))
                    if q == 1:
                        load_consts_late()
                steps.append((0.0, load_x))

            for grp in range(3):
                for e in range(NE):
                    def mm(q=q, grp=grp, e=e):
                        if e == 0:
                            live[("acc", q, grp)] = ps.tile(
                                [128, TQ], F32, tag="acc", bufs=2,
                                name=f"acc{q}_{grp}")
                        acc = live[("acc", q, grp)]
                        if grp < 2:
                            lhs = wq_res[:, e, grp * 128:(grp + 1) * 128]
                        else:
                            lhs = wkv_res[:, e, :]
                        nc.tensor.matmul(acc, lhs, live[("xt", q)][:, e, :],
                                         start=(e == 0), stop=(e == NE - 1))
                    steps.append((213.0, mm))

                def stage(q=q, grp=grp, cs=cs, ce=ce):
                    acc = live.pop(("acc", q, grp))
                    if grp < 2:
                        nc.vector.tensor_copy(
                            out=qTa[2 * grp][0:64, cs:ce], in_=acc[0:64, :])
                        nc.vector.tensor_copy(
                            out=qTa[2 * grp + 1][0:64, cs:ce],
                            in_=acc[64:128, :])
                    else:
                        nc.vector.tensor_copy(
                            out=kTa[0:64, cs:ce], in_=acc[0:64, :])
                        stv = sp.tile([64, TQ], BF16, tag="stv",
                                      name=f"stv{q}")
                        nc.vector.tensor_copy(out=stv, in_=acc[64:128, :])
                        for m in range(4):
                            nc.sync.dma_start_transpose(
                                out=v_aug[:, 4 * q + m, 0:64],
                                in_=stv[:, m * 128:(m + 1) * 128])
                steps.append((0.0, stage))
        return steps

    # ---------------- scores + AV stream (interleaved per q) -----------
    def score_steps_hq(q, h):
        steps = []
        for tk in range(4 * q + 4):
            qs = max(q * TQ, tk * 128)
            n = (q + 1) * TQ - qs

            def step(h=h, q=q, tk=tk, qs=qs, n=n):
                st = ps.tile([128, TQ], F32, tag="st", bufs=4,
                             name=f"st{q}_{h}_{tk}")
                g = gp.tile([128, TQ], BF16, tag=f"g{h}_{tk}",
                            name=f"g{q}_{h}_{tk}")
                live[("g", h, tk)] = (q, g)
                nc.tensor.matmul(
                    st[:, 0:n],
                    kTa[:, tk * 128:(tk + 1) * 128],
                    qTa[h][:, qs:qs + n],
                    start=True, stop=True)
                nc.scalar.activation(
                    out=g[:, 0:n], in_=st[:, 0:n], func=EXP,
                    bias=btbl_t[:, h * NT128 + tk:h * NT128 + tk + 1],
                    scale=1.0)
                if tk >= 4 * q:
                    nc.gpsimd.affine_select(
                        out=g[:, 0:128], in_=g[:, 0:128],
                        compare_op=mybir.AluOpType.is_ge,
                        fill=0.0, base=0,
                        pattern=[[1, 128]], channel_multiplier=-1)
            steps.append((n * 0.4167, step))
        return steps

    def av_steps_hq(q, h):
        steps = []
        for j in range(4):
            ntk = 4 * q + j + 1

            def mmj(h=h, q=q, j=j, ntk=ntk):
                if j == 0:
                    live[("av", h)] = ps.tile(
                        [128, 4, 65], F32, tag="av", bufs=2,
                        name=f"av{q}_{h}")
                av = live[("av", h)]
                for tk in range(ntk):
                    qs = max(q * TQ, tk * 128)
                    off = q * TQ + j * 128 - qs
                    gq, g = live[("g", h, tk)]
                    assert gq == q, f"stale g tile {gq} != {q}"
                    nc.tensor.matmul(
                        av[:, j, :],
                        g[:, off:off + 128],
                        v_aug[:, tk, 0:65],
                        start=(tk == 0), stop=(tk == ntk - 1))
            steps.append((ntk * 65 * 0.4167, mmj))

        def norm(h=h, q=q):
            av = live.pop(("av", h))
            rc = sp.tile([128, 4], F32, tag=f"rc{h}", name=f"rc{q}{h}")
            nc.vector.reciprocal(out=rc, in_=av[:, :, 64])
            hsb = sp.tile([128, 4, 128], BF16, tag=f"hsb{h}",
                          name=f"hsb{q}{h}")
            rc_b = bass.AP(tensor=rc.tensor, offset=rc.offset,
                           ap=[rc.ap[0], [1, 4], [0, 64]])
            nc.vector.tensor_mul(
                out=hsb[:, :, 0:64], in0=av[:, :, 0:64], in1=rc_b)
            # xbar: otn[pair][64*(h%2)+d, q*TQ + m*128 + p] = hsb[p, m, d]
            osl = otn[h // 2][(h % 2) * 64:(h % 2) * 64 + 64,
                              q * TQ:(q + 1) * TQ]
            oap = bass.AP(tensor=osl.tensor, offset=osl.offset,
                          ap=[osl.ap[0], [128, 4], [1, 128]])
            nc.sync.dma_start_transpose(out=oap, in_=hsb[:, :, :])
        steps.append((0.0, norm))
        return steps

    def sa_steps(wo):
        """Per q block: W(q-1) interleaved into the leading score work (it
        is the PE filler while the previous block's norm chains drain and
        this block's exps warm up), then AV lagged ~2.5 heads behind."""
        steps = []
        for q in range(NTQ):
            s = [score_steps_hq(q, h) for h in range(HL)]
            a = [av_steps_hq(q, h) for h in range(HL)]
            head = s[1] + s[2]
            if q > 0:
                merged = []
                _merge([(head, 1.0), (wo[q - 1], 1.0)], collect=merged)
                head = merged
            steps += s[0] + head + a[0] + s[3] + a[1] + a[2] + a[3]
        steps += wo[NTQ - 1]
        return steps

    # ---------------- Wo stream (per source q-tile) ----------------
    def wo_steps_q(qsrc):
        steps = []
        for t in range(4 * qsrc, 4 * qsrc + 4):
            for o in range(4):
                def mmo(t=t, o=o):
                    if o == 0:
                        live[("ob", t)] = obp.tile(
                            [128, E], BF16, tag="ob", name=f"ob{t}")
                    ob = live[("ob", t)]
                    po = ps.tile([128, TQ], F32, tag="acc", bufs=2,
                                 name=f"po{t}_{o}")
                    nc.tensor.matmul(
                        po, otn[0][:, t * 128:(t + 1) * 128],
                        wo_res[0][:, o * TQ:(o + 1) * TQ],
                        start=True, stop=False)
                    nc.tensor.matmul(
                        po, otn[1][:, t * 128:(t + 1) * 128],
                        wo_res[1][:, o * TQ:(o + 1) * TQ],
                        start=False, stop=True)
                    eng = nc.gpsimd if o % 2 == 0 else nc.vector
                    eng.tensor_copy(out=ob[:, o * TQ:(o + 1) * TQ], in_=po)
                    if o % 2 == 1:
                        half = (o - 1) * TQ
                        nc.sync.dma_start(
                            out=part[t * 128:(t + 1) * 128,
                                     half:half + 2 * TQ],
                            in_=ob[:, half:half + 2 * TQ])
                    if o == 3:
                        live.pop(("ob", t))
                steps.append((427.0, mmo))
        return steps

    sP = proj_steps()
    sW = [wo_steps_q(q) for q in range(NTQ)]
    sSA = sa_steps(sW)

    # bootstrap: chunked const loads + first proj q-tile fully
    load_consts_early()
    nboot = 3 * (NE + 1)
    for _c, fn in sP[:nboot]:
        fn()
    load_consts_late()
    _merge([(sP[nboot:], W_PROJ), (sSA, W_SA)])


def _prepare_in_maps(x, Wq, Wk, Wv, Wo):
    bf = ml_dtypes.bfloat16
    xTn = np.ascontiguousarray(x[0].T).astype(bf)
    wkvTn = np.ascontiguousarray(
        np.concatenate([Wk, Wv], axis=0).T).astype(bf)
    scale = np.float64(D) ** -0.5
    in_maps = []
    for c in range(NCORES):
        sl = slice(c * ES, (c + 1) * ES)
        wqTn = np.ascontiguousarray(
            (Wq[sl, :].astype(np.float64) * scale).T).astype(bf)
        woTn = np.ascontiguousarray(Wo[:, sl].T).astype(bf)
        g = np.arange(c * HL, (c + 1) * HL, dtype=np.float64)
        slopes = np.power(2.0, -8.0 * (g + 1.0) / H)          # [HL]
        i = np.arange(T, dtype=np.float64)
        qaug_n = (-slopes[:, None] * i[None, :]).astype(bf)   # [HL, T]
        p = np.arange(128, dtype=np.float64)
        k = np.arange(NT128, dtype=np.float64)
        btbl_n = (slopes[:, None, None]
                  * (k[None, :, None] * 128 + p[None, None, :]))
        btbl_n = np.ascontiguousarray(
            btbl_n.transpose(2, 0, 1).reshape(128, HL * NT128)
        ).astype(np.float32)
        in_maps.append({
            "xT": xTn, "wqT": wqTn, "wkvT": wkvTn, "woT": woTn,
            "qaug": qaug_n, "ones": np.ones((1, T), dtype=bf),
            "btbl": btbl_n,
        })
    return in_maps


def kernel(x, Wq, Wk, Wv, Wo, attention_mask, _trace=False, _trace_cores=None):
    x = np.asarray(x, dtype=np.float32)
    Wq = np.asarray(Wq, dtype=np.float32)
    Wk = np.asarray(Wk, dtype=np.float32)
    Wv = np.asarray(Wv, dtype=np.float32)
    Wo = np.asarray(Wo, dtype=np.float32)

    if "nc" not in _CACHE:
        _CACHE["nc"] = _build_nc()
    nc = _CACHE["nc"]

    in_maps = _prepare_in_maps(x, Wq, Wk, Wv, Wo)
    kwargs = {}
    if _trace:
        kwargs = {"trace": True, "trace_cores": _trace_cores or [0]}
    res = run_bass_kernel_spmd(nc, in_maps, core_ids=list(range(NCORES)),
                               **kwargs)
    acc = np.zeros((T, E), dtype=np.float32)
    for r in res.results:
        acc += np.asarray(r["part"]).astype(np.float32)
    out = acc[None, :, :]
    if _trace:
        _CACHE["last_result"] = res
    return out
